# revision 2
# baseline (speedup 1.0000x reference)
"""GNN message-passing node model on 8 TRN2 NeuronCores — v2.

Reference computation:
    agg = segment_sum(edge_attr, edge_index[1], num_segments=N)   # scatter-add
    h   = relu(concat([x, agg], 1) @ W1 + b1)
    out = h @ W2 + b2

Design (vs the N=128 identity-packed baseline):
  * Nodes are globally degree-sorted and dealt into 98 blocks of 512
    (window w, 64 nodes per core per window); position i in a block goes
    to core i%8, window-rel i//8.  Every (core, window) then holds an
    almost-identical degree multiset, so constant scatter patterns pack
    edges with ~2% padding and the per-core edge totals balance.
    Window order is REVERSED (lowest degree first) so the kernel ramps
    on small groups while the DMA stream builds depth.
  * Seg-sum runs as TensorE matmuls with *64-column* windows:
    aggT[128 feat, 64 nodes] += E_tile[128 edge, 128 feat]^T @ S.
    S is one of two CONSTANT fp8 matrices (no DVE one-hot building):
      - id2 [128, 64]: partition p -> node p//2 (2 slots/node/tile)
      - I128 straddle tiles spanning a window PAIR for odd leftovers.
    64-col matmuls cost ~29ns for up to 128 edges (fp8 FWL LDWEIGHTS
    hides fully) — half the baseline's per-edge TensorE cost.
  * The Tile scheduler serializes each group as [seg, W1, ACT, W2]
    (its DMA cost model believes edges arrive late), so the MLP is
    emitted at PAIR granularity (128-col chunks): W1a x4, W1b x4,
    per-pair RELUs overlap the remaining W1s, W2 x4 — no cross-engine
    bubble on the tensor path.  PSUM->SBUF casts are also per pair so
    W1b's input is ready when the seg matmuls end.
  * Output is computed TRANSPOSED (outT = W2^T @ h, W2 stationary) so
    stores are contiguous 1KB/partition lines; most stores ride the
    gpsimd SWDGE queue (separate completion-sem pool -> no HWDGE lane
    pollution), the last three ride the scalar HWDGE ring.
  * DMA: id2|i128|edges (fp8 e3m4) in one stream + x (fp8) on the sync
    HWDGE ring (~0.6-1.2MB descriptors: small ones are lane-turnaround
    limited); weights + biases + late stores on the scalar ring.
"""

import os
import sys
import types

import numpy as np
import ml_dtypes

N_NODES = 50000
N_EDGES = 600000
H = 128
N_CORES = 8
WIN = 64                          # destination-node window (matmul N dim)
NW = 98                           # windows per core -> 6272 padded nodes
NPC = N_NODES // N_CORES          # 6250
NPAD = NW * WIN                   # 6272
GROUP = 8                         # windows per MLP group (512 cols)
NPAIR = NW // 2                   # 49 window pairs (straddle scope)
SHEAD = 192                       # id2 (64) + I128 (128) cols at edges head


def _install_axon_trace_shim():
    """If the harness sets BASS_TRACE=1, run_bass_kernel_spmd imports
    antenv.axon_hooks; slim axon containers lack it.  Provide the same
    ctypes-based NTFF hook trn_agent_boot would register, so tracing works
    instead of crashing.  No-op when the real module exists."""
    try:
        import antenv.axon_hooks  # noqa: F401
        return
    except ImportError:
        pass
    mod = types.ModuleType("antenv.axon_hooks")
    mod._hook = None
    mod.set_axon_ntff_profile_hook = lambda h: setattr(mod, "_hook", h)
    mod.get_axon_ntff_profile_hook = lambda: mod._hook
    sys.modules["antenv.axon_hooks"] = mod
    so_path = "/opt/axon/libaxon_pjrt.so"
    if os.path.exists(so_path):
        try:
            from trn_agent_boot.trn_boot import _ntff_profile_via_ctypes
            mod._hook = _ntff_profile_via_ctypes(so_path)
        except Exception:
            mod._hook = None
    try:
        from concourse import bass_utils
        _orig_upload = bass_utils.upload_artifacts

        def _safe_upload(tmpdir):
            try:
                return _orig_upload(tmpdir)
            except Exception as e:  # no bucket access in sandbox
                return f"upload-skipped({e.__class__.__name__})"

        bass_utils.upload_artifacts = _safe_upload
    except Exception:
        pass


def _plan(deg):
    """Degree-sorted node permutation + per-pair tile plan."""
    order = np.argsort(-deg, kind="stable")
    node_core = np.empty(N_NODES, np.int64)
    node_w = np.empty(N_NODES, np.int64)
    node_rel = np.empty(N_NODES, np.int64)
    pos = np.arange(N_NODES)
    blk = pos // (N_CORES * WIN)
    ip = pos % (N_CORES * WIN)
    node_core[order] = ip % N_CORES
    # window order REVERSED: lowest-degree windows first
    node_w[order] = NW - 1 - blk
    node_rel[order] = ip // N_CORES

    dmax = np.zeros(NW, np.int64)
    np.maximum.at(dmax, node_w, deg)

    caps_c = np.zeros(NW, np.int64)
    caps_s = np.zeros(NPAIR, np.int64)
    for j in range(NPAIR):
        dA, dB = int(dmax[2 * j]), int(dmax[2 * j + 1])
        best = None
        for s in range(max(dA, dB) + 1):
            cA = max((dA - s + 1) // 2, 1)
            cB = max((dB - s + 1) // 2, 1)
            tiles = cA + cB + s
            mm = cA + cB + 2 * s
            key = (tiles, mm, s)
            if best is None or key < best[0]:
                best = (key, cA, cB, s)
        _, cA, cB, s = best
        caps_c[2 * j], caps_c[2 * j + 1], caps_s[j] = cA, cB, s
    pair_t0 = np.zeros(NPAIR + 1, np.int64)
    for j in range(NPAIR):
        pair_t0[j + 1] = pair_t0[j] + caps_s[j] + caps_c[2*j] + caps_c[2*j+1]
    return node_core, node_w, node_rel, caps_c, caps_s, pair_t0, int(pair_t0[-1])


def _prep_host(x, edge_index, edge_attr, W1, b1, W2, b2, np_edt):
    bf16 = ml_dtypes.bfloat16
    col = np.asarray(edge_index)[1].astype(np.int64)
    deg = np.bincount(col, minlength=N_NODES)
    node_core, node_w, node_rel, caps_c, caps_s, pair_t0, T = _plan(deg)

    # per-edge slot assignment (vectorized, all cores at once)
    so = np.argsort(col, kind="stable")           # edges grouped by node
    scol = col[so]
    node_start = np.concatenate([[0], np.cumsum(deg)[:-1]])
    rank = np.arange(N_EDGES, dtype=np.int64) - node_start[scol]
    e_core = node_core[scol]
    e_w = node_w[scol]
    e_rel = node_rel[scol]
    e_j = e_w // 2
    cw = caps_c[e_w]
    is_id = rank < 2 * cw
    base_id = pair_t0[e_j] + caps_s[e_j] + np.where(e_w % 2 == 1,
                                                    caps_c[2 * e_j], 0)
    tile = np.empty(N_EDGES, np.int64)
    part = np.empty(N_EDGES, np.int64)
    tile[is_id] = base_id[is_id] + rank[is_id] // 2
    part[is_id] = 2 * e_rel[is_id] + rank[is_id] % 2
    ov = ~is_id
    k = rank[ov] - 2 * cw[ov]
    tile[ov] = pair_t0[e_j[ov]] + k
    part[ov] = e_rel[ov] + WIN * (e_w[ov] % 2)

    ea = np.asarray(edge_attr, np.float32).astype(np_edt)[so]
    x = np.asarray(x, np.float32)
    W1 = np.asarray(W1, np.float32)
    b1 = np.asarray(b1, np.float32)
    W2 = np.asarray(W2, np.float32)
    b2 = np.asarray(b2, np.float32)

    id2 = np.zeros((128, WIN), np.float32)
    id2[np.arange(128), np.arange(128) // 2] = 1.0
    i128 = np.eye(128, dtype=np.float32)
    w1a = np.ascontiguousarray(W1[:H]).astype(bf16)
    w1b = np.ascontiguousarray(W1[H:]).astype(bf16)
    w2c = W2.astype(bf16)
    cbf = np.concatenate([id2.astype(bf16), i128.astype(bf16),
                          w1a, w1b, w2c], axis=1)        # [128, 576]
    b1c = np.ascontiguousarray(b1.reshape(H, 1))         # f32, ACT bias
    # fold b2 into hT: out = hT.T@W2 + b2 == (hT + c 1^T).T @ W2 with
    # c = W2^-T b2 (exact; c == 0 when b2 == 0, letting the add be elided)
    try:
        cvec = np.linalg.solve(W2.T.astype(np.float64),
                               b2.astype(np.float64)).astype(np.float32)
    except np.linalg.LinAlgError:
        cvec = np.linalg.lstsq(W2.T.astype(np.float64),
                               b2.astype(np.float64), rcond=None)[0].astype(
                                   np.float32)
    c_zero = bool(np.all(b2 == 0.0)) or bool(np.all(cvec == 0.0))
    cf32 = np.concatenate([b1c, cvec.reshape(H, 1)], axis=1)  # [128, 2]

    in_maps = []
    for c in range(N_CORES):
        m = (e_core == c)
        tiles_c = np.zeros((128, T, H), dtype=np_edt)
        tiles_c[part[m], tile[m], :] = ea[m]
        edges_c = tiles_c.reshape(128, T * H)
        xT_c = np.zeros((H, NPAD), dtype=np_edt)   # x rides fp8 too
        mask = node_core == c
        cols = node_w[mask] * WIN + node_rel[mask]
        xT_c[:, cols] = x[mask].T.astype(np_edt)
        in_maps.append({"edges": edges_c, "cf32": cf32, "xT": xT_c,
                        "cbf": cbf})
    return (in_maps, caps_c.tolist(), caps_s.tolist(), pair_t0.tolist(), T,
            c_zero, node_core, node_w, node_rel)


def _build_program(caps_c, caps_s, pair_t0, T, c_zero, e_dt):
    import concourse.tile as tile
    from concourse import bacc, mybir
    from contextlib import ExitStack

    f32 = mybir.dt.float32
    bf16 = mybir.dt.bfloat16
    nc = bacc.Bacc("TRN2", target_bir_lowering=False, debug=False,
                   num_devices=N_CORES)

    CBF_COLS = WIN + 4 * 128      # 576
    edges_ap = nc.dram_tensor("edges", [128, T * H], e_dt,
                              kind="ExternalInput").ap()
    cf32_ap = nc.dram_tensor("cf32", [128, 2], f32, kind="ExternalInput").ap()
    xT_ap = nc.dram_tensor("xT", [H, NPAD], e_dt, kind="ExternalInput").ap()
    cbf_ap = nc.dram_tensor("cbf", [128, CBF_COLS], bf16,
                            kind="ExternalInput").ap()
    out_dt = bf16  # bf16 store halves output DMA; host casts back to f32
    outT_ap = nc.dram_tensor("outT", [H, NPAD], out_dt,
                             kind="ExternalOutput").ap()

    n_groups = (NW + GROUP - 1) // GROUP          # 13
    PPG = GROUP // 2                              # pairs per full group

    with tile.TileContext(nc) as tc, ExitStack() as ctx:
        const = ctx.enter_context(tc.tile_pool(name="const", bufs=1))
        epool = ctx.enter_context(tc.tile_pool(name="edges", bufs=n_groups))
        apool = ctx.enter_context(tc.tile_pool(name="agg", bufs=4))
        hpool = ctx.enter_context(tc.tile_pool(name="h", bufs=3))
        opool = ctx.enter_context(tc.tile_pool(name="osb", bufs=8))
        # PSUM is the overlap unit: PE-write + engine-read on the SAME
        # bank serialize (Tile's tracker is bank-aware), and start=True
        # clears has_written for the whole bank.  So each group's work is
        # split into TWO half-group banks: cast/ACT/W2 of half A overlap
        # the tensor ops of half B.  4 + 2 + 2 = 8 banks exactly.
        pw = ctx.enter_context(tc.tile_pool(name="pw", bufs=4, space="PSUM"))
        ph = ctx.enter_context(tc.tile_pool(name="ph", bufs=2, space="PSUM"))
        po = ctx.enter_context(tc.tile_pool(name="po", bufs=2, space="PSUM"))

        # cbf (scatter consts + weights, bf16) leads the SYNC ring: on the
        # scalar ring it would sit behind the ~1.3us ACT_TABLE_LOAD and
        # delay the first matmul.  cf32 rides the idle scalar ring.
        cbf_t = const.tile([128, CBF_COLS], bf16)
        nc.sync.dma_start(cbf_t[:], cbf_ap[:])
        cf32_t = const.tile([128, 2], f32)
        nc.scalar.dma_start(cf32_t[:], cf32_ap[:])
        id2_t = cbf_t[:, 0:WIN]
        i128_t = cbf_t[:, WIN:WIN + 128]
        w1a_t = cbf_t[:, WIN + 128:WIN + 256]
        w1b_t = cbf_t[:, WIN + 256:WIN + 384]
        w2_t = cbf_t[:, WIN + 384:WIN + 512]
        b1_t = cf32_t[:, 0:1]
        c_t = cf32_t[:, 1:2]
        xT_t = const.tile([H, NPAD], e_dt)

        # group metadata
        gmeta = []
        for g in range(n_groups):
            plist = list(range(g * PPG, min((g + 1) * PPG, NPAIR)))
            gt0 = pair_t0[plist[0]]
            gt1 = pair_t0[plist[-1] + 1]
            gmeta.append((plist, gt0, gt1))

        # edge prefetch on the sync ring.  Group 0's tile carries the fp8
        # scatter constants (id2|I128) at its head so the very first
        # descriptor delivers everything the first matmuls need.  x (fp8,
        # 0.8MB) goes as one descriptor after E1 (arrives ~11.5us, first
        # needed ~13us).
        etiles = []
        for g in range(n_groups):
            plist, gt0, gt1 = gmeta[g]
            etile = epool.tile([128, (gt1 - gt0) * H], e_dt, tag="edges")
            etiles.append(etile)
            if g == 0:
                # per-pair chunks so the first MMs start early
                for j in plist:
                    t0, t1 = pair_t0[j], pair_t0[j + 1]
                    nc.sync.dma_start(
                        etile[:, (t0 - gt0) * H:(t1 - gt0) * H],
                        edges_ap[:, t0 * H:t1 * H])
            else:
                nc.sync.dma_start(etile[:],
                                  edges_ap[:, gt0 * H:gt1 * H])
            if g == 1:
                nc.sync.dma_start(xT_t[:], xT_ap[:])

        def gcols(g):
            c0 = g * GROUP * WIN
            return c0, (min((g + 1) * GROUP, NW) - g * GROUP) * WIN

        aggs = [None] * n_groups
        hTs = [None] * n_groups

        def halves(g):
            """[(col0, ncols_h, [pairs])] — half-groups of <=2 pairs."""
            plist, _, _ = gmeta[g]
            out = []
            for i in range(0, len(plist), 2):
                ps = plist[i:i + 2]
                col0 = (2 * ps[0] - g * GROUP) * WIN
                out.append((col0, len(ps) * 2 * WIN, ps))
            return out

        def emit_seg(g):
            plist, gt0, gt1 = gmeta[g]
            etile = etiles[g]
            c0, ncols = gcols(g)
            agg_g = apool.tile([H, ncols], bf16, tag="agg")
            for col0, nch, ps in halves(g):
                pwg = pw.tile([H, 512], f32, tag="pw")   # full bank
                for pi, j in enumerate(ps):
                    wA, wB = 2 * j, 2 * j + 1
                    s, cA, cB = caps_s[j], caps_c[wA], caps_c[wB]
                    t0 = pair_t0[j] - gt0
                    pcol = pi * 2 * WIN
                    ppair = pwg[:, pcol:pcol + 2 * WIN]

                    def lhs(t):
                        a = (t0 + t) * H
                        return etile[:, a:a + H]

                    for t in range(s):
                        nc.tensor.matmul(out=ppair, lhsT=lhs(t), rhs=i128_t,
                                         start=(t == 0), stop=False)
                    off = s
                    for wi, cc in ((0, cA), (1, cB)):
                        psl = pwg[:, pcol + wi * WIN:pcol + (wi + 1) * WIN]
                        for t in range(cc):
                            nc.tensor.matmul(
                                out=psl, lhsT=lhs(off + t), rhs=id2_t,
                                start=(s == 0 and t == 0), stop=(t == cc - 1))
                        off += cc
                # this half-bank's cast overlaps the next half's matmuls
                nc.vector.tensor_copy(agg_g[:, col0:col0 + nch],
                                      pwg[:, :nch])
            aggs[g] = agg_g

        def emit_w1(g):
            c0, ncols = gcols(g)
            hT = hpool.tile([H, ncols], bf16, tag="hT")
            for col0, nch, ps in halves(g):
                phh = ph.tile([H, 512], f32, tag="ph")   # full bank
                nc.tensor.matmul(phh[:, :nch], lhsT=w1a_t,
                                 rhs=xT_t[:, c0 + col0:c0 + col0 + nch],
                                 start=True, stop=False)
                nc.tensor.matmul(phh[:, :nch], lhsT=w1b_t,
                                 rhs=aggs[g][:, col0:col0 + nch],
                                 start=False, stop=True)
                # half A's RELU overlaps half B's W1 matmuls (other bank)
                nc.scalar.activation(out=hT[:, col0:col0 + nch],
                                     in_=phh[:, :nch],
                                     func=mybir.ActivationFunctionType.Relu,
                                     bias=b1_t, scale=1.0)
                if not c_zero:
                    nc.scalar.add(hT[:, col0:col0 + nch],
                                  hT[:, col0:col0 + nch], c_t)
            hTs[g] = hT

        def emit_w2(g):
            c0, ncols = gcols(g)
            osb = opool.tile([H, ncols], out_dt, tag="osb")
            for col0, nch, ps in halves(g):
                poT = po.tile([H, 512], f32, tag="po")   # full bank
                nc.tensor.matmul(poT[:, :nch], lhsT=w2_t,
                                 rhs=hTs[g][:, col0:col0 + nch],
                                 start=True, stop=True)
                nc.vector.tensor_copy(osb[:, col0:col0 + nch], poT[:, :nch])
            # stores: gpsimd SWDGE (separate completion-sem pool -> no
            # HWDGE lane pollution); the LAST THREE ride the scalar HWDGE
            # ring (edge loads done by then) for a fast tail.
            if g >= n_groups - 3:
                nc.scalar.dma_start(outT_ap[:, c0:c0 + ncols], osb[:])
            else:
                nc.gpsimd.dma_start(outT_ap[:, c0:c0 + ncols], osb[:])

        for g in range(n_groups):
            emit_seg(g)
            if g >= 3:
                emit_w2(g - 3)
            if g >= 2:
                emit_w1(g - 2)
        emit_w2(n_groups - 3)
        emit_w1(n_groups - 2)
        emit_w2(n_groups - 2)
        emit_w1(n_groups - 1)
        emit_w2(n_groups - 1)

    nc.finalize()
    return nc


def kernel(x, edge_index, edge_attr, u=None, batch=None, W1=None, b1=None,
           W2=None, b2=None, **_unused):
    _install_axon_trace_shim()
    from concourse import mybir
    from concourse.bass_utils import run_bass_kernel_spmd

    np_edt, e_dt = ml_dtypes.float8_e3m4, mybir.dt.float8e3

    (in_maps, caps_c, caps_s, pair_t0, T, c_zero,
     node_core, node_w, node_rel) = _prep_host(
        x, edge_index, edge_attr, W1, b1, W2, b2, np_edt)
    nc = _build_program(caps_c, caps_s, pair_t0, T, c_zero, e_dt)
    res = run_bass_kernel_spmd(nc, in_maps, core_ids=list(range(N_CORES)))
    out = np.empty((N_NODES, H), np.float32)
    for c in range(N_CORES):
        outT = np.asarray(res.results[c]["outT"], np.float32)   # [H, NPAD]
        mask = node_core == c
        cols = node_w[mask] * WIN + node_rel[mask]
        out[mask] = outT[:, cols].T
    return np.ascontiguousarray(out)
